# revision 3
# baseline (speedup 1.0000x reference)
"""Grouped-expert FFN (MoE) kernel for Trainium2, expert-parallel over 8 NeuronCores.

Problem: x[16,2048,1024] @ w1[16,1024,4096] + b1 -> gelu -> @ w2[16,4096,1024] + b2.

Sharding: expert dim E=16 split as 2 experts per core (x, w1, w2 on axis 0;
b1/b2 replicated). Fully local grouped GEMM per core.

v2 layout: all matmul operands in bf16 (full PE rate, half the HBM traffic and
SBUF footprint of fp32r). The full token dim N=2048 is processed in one pass, so
each weight tile is DMA'd and LD_WEIGHTS-loaded exactly once per expert:
  GEMM1: hiddenT[h,n] = w1[d,h].T @ xT[d,n]   (lhsT = w1 tile, rhs = xT)
  GEMM2: outT[d,n]    = w2[h,d].T @ hiddenT[h,n]
hiddenT (bf16) for all 32 h-tiles x 2048 tokens stays resident in SBUF.

DMA issue is split across engine queues so a throttled stream never blocks an
unrelated prefetch (per-engine DGE queues are strictly in-order):
  SP (sync):    w1 + w2 weight streams
  Pool (gpsimd): x tiles, biases, first two w2 g-blocks of each expert
  ACT (scalar): output stores (each depends only on the act just before it)
PSUM: one shared pool of all 8 banks; phases A/B each keep 4-bank accumulation
groups (one per 512-token chunk), double-buffered across m iterations.
"""

import numpy as np

E_FULL = 16
N_TOK = 2048
D_DIM = 1024
H_DIM = 4096
N_CORES = 8
E_LOC = E_FULL // N_CORES  # 2 experts per core
NB = 512                   # matmul moving-dim chunk (= one PSUM bank of fp32)
NBS = N_TOK // NB          # 4 chunks
KD = D_DIM // 128          # 8  k-tiles for GEMM1
KH = H_DIM // 128          # 32 k-tiles for GEMM2
MH = H_DIM // 128          # 32 m-tiles (hidden rows) for GEMM1
MD = D_DIM // 128          # 8  m-tiles (out rows) for GEMM2
G = KH // 8                # 4  w2 g-blocks of 8 k-tiles each

_CACHE = {}


def _build(bench_iters=None):
    from concourse import bass, tile, mybir, bacc
    from contextlib import nullcontext

    BF16 = mybir.dt.bfloat16
    F32 = mybir.dt.float32
    AF = mybir.ActivationFunctionType

    nc = bacc.Bacc("TRN2", target_bir_lowering=False, debug=False)

    xT = nc.dram_tensor("xT", (E_LOC, D_DIM, N_TOK), BF16, kind="ExternalInput").ap()
    # host-swizzled: w1s[e, m, p, k*128+j] = w1[e, k*128+p, m*128+j]
    w1 = nc.dram_tensor(
        "w1s", (E_LOC, MH, 128, KD * 128), BF16, kind="ExternalInput"
    ).ap()
    # host-swizzled: w2s[e, m2, g, p, ki*128+j] = w2[e, (g*8+ki)*128+p, m2*128+j]
    w2 = nc.dram_tensor(
        "w2s", (E_LOC, MD, G, 128, 8 * 128), BF16, kind="ExternalInput"
    ).ap()
    b1c = nc.dram_tensor("b1c", (128, MH), F32, kind="ExternalInput").ap()
    b2c = nc.dram_tensor("b2c", (128, MD), F32, kind="ExternalInput").ap()
    outT = nc.dram_tensor("outT", (E_LOC, D_DIM, N_TOK), F32, kind="ExternalOutput").ap()

    with tile.TileContext(nc) as tc:
        with (
            tc.tile_pool(name="xp", bufs=12) as xp,
            tc.tile_pool(name="hp", bufs=MH) as hp,
            tc.tile_pool(name="w1p", bufs=4) as w1p,
            tc.tile_pool(name="w2p", bufs=4) as w2p,
            tc.tile_pool(name="w2q", bufs=2) as w2q,
            tc.tile_pool(name="op", bufs=4) as op,
            tc.tile_pool(name="bp", bufs=1) as bp,
            tc.tile_pool(name="ps", bufs=8, space=bass.MemorySpace.PSUM) as ps,
        ):
            loop_cm = (
                tc.For_i(
                    0,
                    bench_iters,
                    1,
                    hint_engines=(
                        mybir.EngineType.PE,
                        mybir.EngineType.Activation,
                        mybir.EngineType.SP,
                        mybir.EngineType.DVE,
                        mybir.EngineType.Pool,
                    ),
                )
                if bench_iters is not None
                else nullcontext()
            )
            with loop_cm:
                b1t = bp.tile([128, MH], F32, tag="b1")
                b2t = bp.tile([128, MD], F32, tag="b2")

                for e in range(E_LOC):
                    # ---- x tiles for this expert: 8 x [128d, 2048n] on Pool.
                    # Two column-half DMAs per tile so phase A's first m
                    # iteration can start on the first half while the second
                    # streams in (matmuls depend on per-region writes).
                    xts = []
                    for k in range(KD):
                        xt = xp.tile([128, N_TOK], BF16, tag="x")
                        nc.gpsimd.dma_start(
                            xt[:, : N_TOK // 2],
                            xT[e, k * 128 : (k + 1) * 128, : N_TOK // 2],
                        )
                        nc.gpsimd.dma_start(
                            xt[:, N_TOK // 2 :],
                            xT[e, k * 128 : (k + 1) * 128, N_TOK // 2 :],
                        )
                        xts.append(xt)
                    if e == 0:
                        # biases are first needed by the m=0 activation, well
                        # after the x stream: keep them off the critical path
                        nc.gpsimd.dma_start(b1t[:], b1c[:])
                        nc.gpsimd.dma_start(b2t[:], b2c[:])
                    # prefetch first two w2 g-blocks of this expert on Pool, so
                    # phase B m2=0 never waits on the SP weight stream
                    wqs = []
                    for g in range(2):
                        wq = w2q.tile([128, 8 * 128], BF16, tag="w2q", name="wq")
                        nc.gpsimd.dma_start(wq[:], w2[e, 0, g])
                        wqs.append(wq)

                    # ---- phase A: hiddenT[h, n] = gelu(w1.T @ xT + b1) ----
                    hts = []
                    for m in range(MH):
                        pa = [
                            ps.tile([128, NB], F32, tag="ps", name=f"pa{_}")
                            for _ in range(NBS)
                        ]
                        wblk = w1p.tile([128, KD * 128], BF16, tag="w1", name="wblk")
                        nc.sync.dma_start(wblk[:], w1[e, m])
                        for k in range(KD):
                            for nb in range(NBS):
                                nc.tensor.matmul(
                                    pa[nb][:],
                                    wblk[:, k * 128 : (k + 1) * 128],
                                    xts[k][:, nb * NB : (nb + 1) * NB],
                                    start=(k == 0),
                                    stop=(k == KD - 1),
                                )
                        ht = hp.tile([128, N_TOK], BF16, tag="h")
                        for nb in range(NBS):
                            nc.scalar.activation(
                                ht[:, nb * NB : (nb + 1) * NB],
                                pa[nb][:],
                                AF.Gelu,
                                bias=b1t[:, m : m + 1],
                            )
                        hts.append(ht)

                    # ---- phase B: outT[d, n] = w2.T @ hiddenT + b2 ----
                    for m2 in range(MD):
                        pb = [
                            ps.tile([128, NB], F32, tag="ps", name=f"pb{_}")
                            for _ in range(NBS)
                        ]
                        for g in range(G):
                            if m2 == 0 and g < 2:
                                wblk2 = wqs[g]
                            else:
                                wblk2 = w2p.tile(
                                    [128, 8 * 128], BF16, tag="w2", name="wblk2"
                                )
                                nc.sync.dma_start(wblk2[:], w2[e, m2, g])
                            for ki in range(8):
                                k = g * 8 + ki
                                for nb in range(NBS):
                                    nc.tensor.matmul(
                                        pb[nb][:],
                                        wblk2[:, ki * 128 : (ki + 1) * 128],
                                        hts[k][:, nb * NB : (nb + 1) * NB],
                                        start=(k == 0),
                                        stop=(k == KH - 1),
                                    )
                        # kernel tail: for the very last m2, drain the four
                        # PSUM chunks on two engines (ACT + DVE, identical f32
                        # bias add) and store via the idle Pool/SP queues, so
                        # the drain doesn't serialize behind ACT SEQ
                        last = e == E_LOC - 1 and m2 == MD - 1
                        for nb in range(NBS):
                            ot = op.tile([128, NB], F32, tag="o")
                            if last and nb % 2 == 1:
                                nc.vector.tensor_scalar_add(
                                    ot[:], pb[nb][:], b2t[:, m2 : m2 + 1]
                                )
                            else:
                                nc.scalar.activation(
                                    ot[:],
                                    pb[nb][:],
                                    AF.Identity,
                                    bias=b2t[:, m2 : m2 + 1],
                                )
                            store_eng = (
                                (nc.gpsimd if nb % 2 == 0 else nc.sync)
                                if last
                                else nc.scalar
                            )
                            store_eng.dma_start(
                                outT[
                                    e,
                                    m2 * 128 : (m2 + 1) * 128,
                                    nb * NB : (nb + 1) * NB,
                                ],
                                ot[:],
                            )

    nc.compile()
    return nc


def get_nc():
    if "nc" not in _CACHE:
        _CACHE["nc"] = _build()
    return _CACHE["nc"]


def _np_bf16():
    from concourse import mybir

    return mybir.dt.np(mybir.dt.bfloat16)


def _swizzle_w1(w1_loc):
    # [E, D, H] -> [E, MH, 128p, KD*128] with w1s[e,m,p,k*128+j] = w1[e,k*128+p,m*128+j]
    e = w1_loc.shape[0]
    v = w1_loc.reshape(e, KD, 128, MH, 128)  # e,k,p,m,j
    return np.ascontiguousarray(v.transpose(0, 3, 2, 1, 4)).reshape(
        e, MH, 128, KD * 128
    )


def _swizzle_w2(w2_loc):
    # [E, H, D] -> [E, MD, G, 128p, 8*128] with w2s[e,m2,g,p,ki*128+j] = w2[e,(g*8+ki)*128+p,m2*128+j]
    e = w2_loc.shape[0]
    v = w2_loc.reshape(e, G, 8, 128, MD, 128)  # e,g,ki,p,m2,j
    return np.ascontiguousarray(v.transpose(0, 4, 1, 3, 2, 5)).reshape(
        e, MD, G, 128, 8 * 128
    )


def make_in_maps(x, w1, w2, b1, b2):
    bf16 = _np_bf16()
    b1c = np.ascontiguousarray(b1.reshape(MH, 128).T, dtype=np.float32)
    b2c = np.ascontiguousarray(b2.reshape(MD, 128).T, dtype=np.float32)
    in_maps = []
    for c in range(N_CORES):
        sl = slice(E_LOC * c, E_LOC * (c + 1))
        in_maps.append(
            {
                "xT": np.ascontiguousarray(x[sl].transpose(0, 2, 1)).astype(bf16),
                "w1s": _swizzle_w1(np.asarray(w1[sl])).astype(bf16),
                "w2s": _swizzle_w2(np.asarray(w2[sl])).astype(bf16),
                "b1c": b1c,
                "b2c": b2c,
            }
        )
    return in_maps


def kernel(x, w1, w2, b1, b2):
    from concourse import bass_utils

    nc = get_nc()
    in_maps = make_in_maps(x, w1, w2, b1, b2)
    res = bass_utils.run_bass_kernel_spmd(nc, in_maps, core_ids=list(range(N_CORES)))
    out = np.empty((E_FULL, N_TOK, D_DIM), dtype=np.float32)
    for c in range(N_CORES):
        out[E_LOC * c : E_LOC * (c + 1)] = res.results[c]["outT"].transpose(0, 2, 1)
    return out


# revision 4
# speedup vs baseline: 1.0016x; 1.0016x over previous
"""Grouped-expert FFN (MoE) kernel for Trainium2, expert-parallel over 8 NeuronCores.

Problem: x[16,2048,1024] @ w1[16,1024,4096] + b1 -> gelu -> @ w2[16,4096,1024] + b2.

Sharding: expert dim E=16 split as 2 experts per core (x, w1, w2 on axis 0;
b1/b2 replicated). Fully local grouped GEMM per core.

v2 layout: all matmul operands in bf16 (full PE rate, half the HBM traffic and
SBUF footprint of fp32r). The full token dim N=2048 is processed in one pass, so
each weight tile is DMA'd and LD_WEIGHTS-loaded exactly once per expert:
  GEMM1: hiddenT[h,n] = w1[d,h].T @ xT[d,n]   (lhsT = w1 tile, rhs = xT)
  GEMM2: outT[d,n]    = w2[h,d].T @ hiddenT[h,n]
hiddenT (bf16) for all 32 h-tiles x 2048 tokens stays resident in SBUF.

DMA issue is split across engine queues so a throttled stream never blocks an
unrelated prefetch (per-engine DGE queues are strictly in-order):
  SP (sync):    w1 + w2 weight streams
  Pool (gpsimd): x tiles, biases, first two w2 g-blocks of each expert
  ACT (scalar): output stores (each depends only on the act just before it)
PSUM: one shared pool of all 8 banks; phases A/B each keep 4-bank accumulation
groups (one per 512-token chunk), double-buffered across m iterations.
"""

import numpy as np

E_FULL = 16
N_TOK = 2048
D_DIM = 1024
H_DIM = 4096
N_CORES = 8
E_LOC = E_FULL // N_CORES  # 2 experts per core
NB = 512                   # matmul moving-dim chunk (= one PSUM bank of fp32)
NBS = N_TOK // NB          # 4 chunks
KD = D_DIM // 128          # 8  k-tiles for GEMM1
KH = H_DIM // 128          # 32 k-tiles for GEMM2
MH = H_DIM // 128          # 32 m-tiles (hidden rows) for GEMM1
MD = D_DIM // 128          # 8  m-tiles (out rows) for GEMM2
G = KH // 8                # 4  w2 g-blocks of 8 k-tiles each

_CACHE = {}


def _build(bench_iters=None):
    from concourse import bass, tile, mybir, bacc
    from contextlib import nullcontext

    BF16 = mybir.dt.bfloat16
    F32 = mybir.dt.float32
    AF = mybir.ActivationFunctionType

    nc = bacc.Bacc("TRN2", target_bir_lowering=False, debug=False)

    xT = nc.dram_tensor("xT", (E_LOC, D_DIM, N_TOK), BF16, kind="ExternalInput").ap()
    # host-swizzled: w1s[e, m, p, k*128+j] = w1[e, k*128+p, m*128+j]
    w1 = nc.dram_tensor(
        "w1s", (E_LOC, MH, 128, KD * 128), BF16, kind="ExternalInput"
    ).ap()
    # host-swizzled: w2s[e, m2, g, p, ki*128+j] = w2[e, (g*8+ki)*128+p, m2*128+j]
    w2 = nc.dram_tensor(
        "w2s", (E_LOC, MD, G, 128, 8 * 128), BF16, kind="ExternalInput"
    ).ap()
    b1c = nc.dram_tensor("b1c", (128, MH), F32, kind="ExternalInput").ap()
    b2c = nc.dram_tensor("b2c", (128, MD), F32, kind="ExternalInput").ap()
    outT = nc.dram_tensor("outT", (E_LOC, D_DIM, N_TOK), F32, kind="ExternalOutput").ap()

    with tile.TileContext(nc) as tc:
        with (
            tc.tile_pool(name="xp", bufs=12) as xp,
            tc.tile_pool(name="hp", bufs=MH) as hp,
            tc.tile_pool(name="w1p", bufs=4) as w1p,
            tc.tile_pool(name="w2p", bufs=4) as w2p,
            tc.tile_pool(name="w2q", bufs=2) as w2q,
            tc.tile_pool(name="op", bufs=4) as op,
            tc.tile_pool(name="bp", bufs=1) as bp,
            tc.tile_pool(name="ps", bufs=8, space=bass.MemorySpace.PSUM) as ps,
        ):
            loop_cm = (
                tc.For_i(
                    0,
                    bench_iters,
                    1,
                    hint_engines=(
                        mybir.EngineType.PE,
                        mybir.EngineType.Activation,
                        mybir.EngineType.SP,
                        mybir.EngineType.DVE,
                        mybir.EngineType.Pool,
                    ),
                )
                if bench_iters is not None
                else nullcontext()
            )
            with loop_cm:
                b1t = bp.tile([128, MH], F32, tag="b1")
                b2t = bp.tile([128, MD], F32, tag="b2")

                for e in range(E_LOC):
                    # ---- x tiles for this expert: 8 x [128d, 2048n] on Pool.
                    # Two column-half DMAs per tile so phase A's first m
                    # iteration can start on the first half while the second
                    # streams in (matmuls depend on per-region writes).
                    xts = []
                    for k in range(KD):
                        xt = xp.tile([128, N_TOK], BF16, tag="x")
                        nc.gpsimd.dma_start(
                            xt[:, : N_TOK // 2],
                            xT[e, k * 128 : (k + 1) * 128, : N_TOK // 2],
                        )
                        nc.gpsimd.dma_start(
                            xt[:, N_TOK // 2 :],
                            xT[e, k * 128 : (k + 1) * 128, N_TOK // 2 :],
                        )
                        xts.append(xt)
                    if e == 0:
                        # biases are first needed by the m=0 activation, well
                        # after the x stream: keep them off the critical path
                        nc.gpsimd.dma_start(b1t[:], b1c[:])
                        nc.gpsimd.dma_start(b2t[:], b2c[:])
                    # prefetch first two w2 g-blocks of this expert on Pool, so
                    # phase B m2=0 never waits on the SP weight stream
                    wqs = []
                    for g in range(2):
                        wq = w2q.tile([128, 8 * 128], BF16, tag="w2q", name="wq")
                        nc.gpsimd.dma_start(wq[:], w2[e, 0, g])
                        wqs.append(wq)

                    # ---- phase A: hiddenT[h, n] = gelu(w1.T @ xT + b1) ----
                    hts = []
                    for m in range(MH):
                        pa = [
                            ps.tile([128, NB], F32, tag="ps", name=f"pa{_}")
                            for _ in range(NBS)
                        ]
                        wblk = w1p.tile([128, KD * 128], BF16, tag="w1", name="wblk")
                        nc.sync.dma_start(wblk[:], w1[e, m])
                        for k in range(KD):
                            for nb in range(NBS):
                                nc.tensor.matmul(
                                    pa[nb][:],
                                    wblk[:, k * 128 : (k + 1) * 128],
                                    xts[k][:, nb * NB : (nb + 1) * NB],
                                    start=(k == 0),
                                    stop=(k == KD - 1),
                                )
                        ht = hp.tile([128, N_TOK], BF16, tag="h")
                        for nb in range(NBS):
                            nc.scalar.activation(
                                ht[:, nb * NB : (nb + 1) * NB],
                                pa[nb][:],
                                AF.Gelu,
                                bias=b1t[:, m : m + 1],
                            )
                        hts.append(ht)

                    # ---- phase B: outT[d, n] = w2.T @ hiddenT + b2 ----
                    for m2 in range(MD):
                        pb = [
                            ps.tile([128, NB], F32, tag="ps", name=f"pb{_}")
                            for _ in range(NBS)
                        ]
                        for g in range(G):
                            if m2 == 0 and g < 2:
                                wblk2 = wqs[g]
                            else:
                                wblk2 = w2p.tile(
                                    [128, 8 * 128], BF16, tag="w2", name="wblk2"
                                )
                                nc.sync.dma_start(wblk2[:], w2[e, m2, g])
                            for ki in range(8):
                                k = g * 8 + ki
                                for nb in range(NBS):
                                    nc.tensor.matmul(
                                        pb[nb][:],
                                        wblk2[:, ki * 128 : (ki + 1) * 128],
                                        hts[k][:, nb * NB : (nb + 1) * NB],
                                        start=(k == 0),
                                        stop=(k == KH - 1),
                                    )
                        # kernel tail: for the very last m2, drain the four
                        # PSUM chunks on two engines (ACT + DVE, identical f32
                        # bias add) and store via the idle Pool/SP queues, so
                        # the drain doesn't serialize behind ACT SEQ
                        last = e == E_LOC - 1 and m2 == MD - 1
                        for nb in range(NBS):
                            ot = op.tile([128, NB], F32, tag="o")
                            if last and nb % 2 == 1:
                                nc.vector.tensor_scalar_add(
                                    ot[:], pb[nb][:], b2t[:, m2 : m2 + 1]
                                )
                            else:
                                nc.scalar.activation(
                                    ot[:],
                                    pb[nb][:],
                                    AF.Identity,
                                    bias=b2t[:, m2 : m2 + 1],
                                )
                            store_eng = (
                                (nc.gpsimd if nb % 2 == 0 else nc.sync)
                                if last
                                else nc.scalar
                            )
                            store_eng.dma_start(
                                outT[
                                    e,
                                    m2 * 128 : (m2 + 1) * 128,
                                    nb * NB : (nb + 1) * NB,
                                ],
                                ot[:],
                            )

    nc.compile()
    return nc


def get_nc():
    if "nc" not in _CACHE:
        _CACHE["nc"] = _build()
    return _CACHE["nc"]


def _np_bf16():
    from concourse import mybir

    return mybir.dt.np(mybir.dt.bfloat16)


def _swizzle_w1(w1_loc):
    # [E, D, H] -> [E, MH, 128p, KD*128] with w1s[e,m,p,k*128+j] = w1[e,k*128+p,m*128+j]
    e = w1_loc.shape[0]
    v = w1_loc.reshape(e, KD, 128, MH, 128)  # e,k,p,m,j
    return np.ascontiguousarray(v.transpose(0, 3, 2, 1, 4)).reshape(
        e, MH, 128, KD * 128
    )


def _swizzle_w2(w2_loc):
    # [E, H, D] -> [E, MD, G, 128p, 8*128] with w2s[e,m2,g,p,ki*128+j] = w2[e,(g*8+ki)*128+p,m2*128+j]
    e = w2_loc.shape[0]
    v = w2_loc.reshape(e, G, 8, 128, MD, 128)  # e,g,ki,p,m2,j
    return np.ascontiguousarray(v.transpose(0, 4, 1, 3, 2, 5)).reshape(
        e, MD, G, 128, 8 * 128
    )


def make_in_maps(x, w1, w2, b1, b2):
    bf16 = _np_bf16()
    x = np.asarray(x)
    w1 = np.asarray(w1)
    w2 = np.asarray(w2)
    b1 = np.asarray(b1)
    b2 = np.asarray(b2)
    b1c = np.ascontiguousarray(b1.reshape(MH, 128).T, dtype=np.float32)
    b2c = np.ascontiguousarray(b2.reshape(MD, 128).T, dtype=np.float32)
    in_maps = []
    for c in range(N_CORES):
        sl = slice(E_LOC * c, E_LOC * (c + 1))
        in_maps.append(
            {
                "xT": np.ascontiguousarray(x[sl].transpose(0, 2, 1)).astype(bf16),
                "w1s": _swizzle_w1(np.asarray(w1[sl])).astype(bf16),
                "w2s": _swizzle_w2(np.asarray(w2[sl])).astype(bf16),
                "b1c": b1c,
                "b2c": b2c,
            }
        )
    return in_maps


def kernel(x, w1, w2, b1, b2):
    from concourse import bass_utils

    nc = get_nc()
    in_maps = make_in_maps(x, w1, w2, b1, b2)
    res = bass_utils.run_bass_kernel_spmd(nc, in_maps, core_ids=list(range(N_CORES)))
    out = np.empty((E_FULL, N_TOK, D_DIM), dtype=np.float32)
    for c in range(N_CORES):
        out[E_LOC * c : E_LOC * (c + 1)] = res.results[c]["outT"].transpose(0, 2, 1)
    return out


# revision 6
# speedup vs baseline: 1.0022x; 1.0006x over previous
"""Grouped-expert FFN (MoE) kernel for Trainium2, expert-parallel over 8 NeuronCores.

Problem: x[16,2048,1024] @ w1[16,1024,4096] + b1 -> gelu -> @ w2[16,4096,1024] + b2.

Sharding: expert dim E=16 split as 2 experts per core (x, w1, w2 on axis 0;
b1/b2 replicated). Fully local grouped GEMM per core.

v2 layout: all matmul operands in bf16 (full PE rate, half the HBM traffic and
SBUF footprint of fp32r). The full token dim N=2048 is processed in one pass, so
each weight tile is DMA'd and LD_WEIGHTS-loaded exactly once per expert:
  GEMM1: hiddenT[h,n] = w1[d,h].T @ xT[d,n]   (lhsT = w1 tile, rhs = xT)
  GEMM2: outT[d,n]    = w2[h,d].T @ hiddenT[h,n]
hiddenT (bf16) for all 32 h-tiles x 2048 tokens stays resident in SBUF.

DMA issue is split across engine queues so a throttled stream never blocks an
unrelated prefetch (per-engine DGE queues are strictly in-order):
  SP (sync):    w1 + w2 weight streams
  Pool (gpsimd): x tiles, biases, first two w2 g-blocks of each expert
  ACT (scalar): output stores (each depends only on the act just before it)
PSUM: one shared pool of all 8 banks; phases A/B each keep 4-bank accumulation
groups (one per 512-token chunk), double-buffered across m iterations.
"""

import numpy as np

E_FULL = 16
N_TOK = 2048
D_DIM = 1024
H_DIM = 4096
N_CORES = 8
E_LOC = E_FULL // N_CORES  # 2 experts per core
NB = 512                   # matmul moving-dim chunk (= one PSUM bank of fp32)
NBS = N_TOK // NB          # 4 chunks
KD = D_DIM // 128          # 8  k-tiles for GEMM1
KH = H_DIM // 128          # 32 k-tiles for GEMM2
MH = H_DIM // 128          # 32 m-tiles (hidden rows) for GEMM1
MD = D_DIM // 128          # 8  m-tiles (out rows) for GEMM2
G = KH // 8                # 4  w2 g-blocks of 8 k-tiles each

_CACHE = {}


def _build(bench_iters=None):
    from concourse import bass, tile, mybir, bacc
    from contextlib import nullcontext

    BF16 = mybir.dt.bfloat16
    F32 = mybir.dt.float32
    AF = mybir.ActivationFunctionType

    nc = bacc.Bacc("TRN2", target_bir_lowering=False, debug=False)

    xT = nc.dram_tensor("xT", (E_LOC, D_DIM, N_TOK), BF16, kind="ExternalInput").ap()
    # host-swizzled: w1s[e, m, p, k*128+j] = w1[e, k*128+p, m*128+j]
    w1 = nc.dram_tensor(
        "w1s", (E_LOC, MH, 128, KD * 128), BF16, kind="ExternalInput"
    ).ap()
    # host-swizzled: w2s[e, m2, g, p, ki*128+j] = w2[e, (g*8+ki)*128+p, m2*128+j]
    w2 = nc.dram_tensor(
        "w2s", (E_LOC, MD, G, 128, 8 * 128), BF16, kind="ExternalInput"
    ).ap()
    b1c = nc.dram_tensor("b1c", (128, MH), F32, kind="ExternalInput").ap()
    b2c = nc.dram_tensor("b2c", (128, MD), F32, kind="ExternalInput").ap()
    outT = nc.dram_tensor("outT", (E_LOC, D_DIM, N_TOK), F32, kind="ExternalOutput").ap()

    with tile.TileContext(nc) as tc:
        with (
            tc.tile_pool(name="xp", bufs=10) as xp,
            tc.tile_pool(name="hp", bufs=MH) as hp,
            tc.tile_pool(name="w1p", bufs=6) as w1p,
            tc.tile_pool(name="w2p", bufs=6) as w2p,
            tc.tile_pool(name="w2q", bufs=2) as w2q,
            tc.tile_pool(name="op", bufs=4) as op,
            tc.tile_pool(name="bp", bufs=1) as bp,
            tc.tile_pool(name="ps", bufs=8, space=bass.MemorySpace.PSUM) as ps,
        ):
            loop_cm = (
                tc.For_i(
                    0,
                    bench_iters,
                    1,
                    hint_engines=(
                        mybir.EngineType.PE,
                        mybir.EngineType.Activation,
                        mybir.EngineType.SP,
                        mybir.EngineType.DVE,
                        mybir.EngineType.Pool,
                    ),
                )
                if bench_iters is not None
                else nullcontext()
            )
            with loop_cm:
                b1t = bp.tile([128, MH], F32, tag="b1")
                b2t = bp.tile([128, MD], F32, tag="b2")
                # warmup: trigger the ACT engine's Gelu table load
                # (LoadActFuncSet, ~1.3us) at t=0, off the critical path --
                # otherwise it delays the first PSUM drain of phase A
                warm = bp.tile([128, 1], F32, tag="warm")
                nc.scalar.activation(warm[:], warm[:], AF.Gelu)

                for e in range(E_LOC):
                    # ---- x tiles for this expert: 8 x [128d, 2048n] on Pool.
                    # Two column-half DMAs per tile so phase A's first m
                    # iteration can start on the first half while the second
                    # streams in (matmuls depend on per-region writes).
                    xts = []
                    for k in range(KD):
                        xt = xp.tile([128, N_TOK], BF16, tag="x")
                        nc.gpsimd.dma_start(
                            xt[:, : N_TOK // 2],
                            xT[e, k * 128 : (k + 1) * 128, : N_TOK // 2],
                        )
                        nc.gpsimd.dma_start(
                            xt[:, N_TOK // 2 :],
                            xT[e, k * 128 : (k + 1) * 128, N_TOK // 2 :],
                        )
                        xts.append(xt)
                    if e == 0:
                        # biases are first needed by the m=0 activation, well
                        # after the x stream: keep them off the critical path
                        nc.gpsimd.dma_start(b1t[:], b1c[:])
                        nc.gpsimd.dma_start(b2t[:], b2c[:])
                    # prefetch first two w2 g-blocks of this expert on Pool, so
                    # phase B m2=0 never waits on the SP weight stream
                    wqs = []
                    for g in range(2):
                        wq = w2q.tile([128, 8 * 128], BF16, tag="w2q", name="wq")
                        nc.gpsimd.dma_start(wq[:], w2[e, 0, g])
                        wqs.append(wq)

                    # ---- phase A: hiddenT[h, n] = gelu(w1.T @ xT + b1) ----
                    hts = []
                    for m in range(MH):
                        pa = [
                            ps.tile([128, NB], F32, tag="ps", name=f"pa{_}")
                            for _ in range(NBS)
                        ]
                        wblk = w1p.tile([128, KD * 128], BF16, tag="w1", name="wblk")
                        nc.sync.dma_start(wblk[:], w1[e, m])
                        for k in range(KD):
                            for nb in range(NBS):
                                nc.tensor.matmul(
                                    pa[nb][:],
                                    wblk[:, k * 128 : (k + 1) * 128],
                                    xts[k][:, nb * NB : (nb + 1) * NB],
                                    start=(k == 0),
                                    stop=(k == KD - 1),
                                )
                        ht = hp.tile([128, N_TOK], BF16, tag="h")
                        for nb in range(NBS):
                            nc.scalar.activation(
                                ht[:, nb * NB : (nb + 1) * NB],
                                pa[nb][:],
                                AF.Gelu,
                                bias=b1t[:, m : m + 1],
                            )
                        hts.append(ht)

                    # ---- phase B: outT[d, n] = w2.T @ hiddenT + b2 ----
                    for m2 in range(MD):
                        pb = [
                            ps.tile([128, NB], F32, tag="ps", name=f"pb{_}")
                            for _ in range(NBS)
                        ]
                        for g in range(G):
                            if m2 == 0 and g < 2:
                                wblk2 = wqs[g]
                            else:
                                wblk2 = w2p.tile(
                                    [128, 8 * 128], BF16, tag="w2", name="wblk2"
                                )
                                nc.sync.dma_start(wblk2[:], w2[e, m2, g])
                            for ki in range(8):
                                k = g * 8 + ki
                                for nb in range(NBS):
                                    nc.tensor.matmul(
                                        pb[nb][:],
                                        wblk2[:, ki * 128 : (ki + 1) * 128],
                                        hts[k][:, nb * NB : (nb + 1) * NB],
                                        start=(k == 0),
                                        stop=(k == KH - 1),
                                    )
                        # kernel tail: for the very last m2, drain the four
                        # PSUM chunks on two engines (ACT + DVE, identical f32
                        # bias add) and store via the idle Pool/SP queues, so
                        # the drain doesn't serialize behind ACT SEQ.
                        # In bench (For_i) mode skip this: a store on Pool/SP
                        # would block the next iteration's x/w prefetch behind
                        # this iteration's final act at every loop boundary.
                        last = (
                            bench_iters is None
                            and e == E_LOC - 1
                            and m2 == MD - 1
                        )
                        for nb in range(NBS):
                            ot = op.tile([128, NB], F32, tag="o")
                            if last and nb % 2 == 1:
                                nc.vector.tensor_scalar_add(
                                    ot[:], pb[nb][:], b2t[:, m2 : m2 + 1]
                                )
                            else:
                                nc.scalar.activation(
                                    ot[:],
                                    pb[nb][:],
                                    AF.Identity,
                                    bias=b2t[:, m2 : m2 + 1],
                                )
                            store_eng = (
                                (nc.gpsimd if nb % 2 == 0 else nc.sync)
                                if last
                                else nc.scalar
                            )
                            store_eng.dma_start(
                                outT[
                                    e,
                                    m2 * 128 : (m2 + 1) * 128,
                                    nb * NB : (nb + 1) * NB,
                                ],
                                ot[:],
                            )

    nc.compile()
    return nc


def get_nc():
    if "nc" not in _CACHE:
        _CACHE["nc"] = _build()
    return _CACHE["nc"]


def _np_bf16():
    from concourse import mybir

    return mybir.dt.np(mybir.dt.bfloat16)


def _swizzle_w1(w1_loc):
    # [E, D, H] -> [E, MH, 128p, KD*128] with w1s[e,m,p,k*128+j] = w1[e,k*128+p,m*128+j]
    e = w1_loc.shape[0]
    v = w1_loc.reshape(e, KD, 128, MH, 128)  # e,k,p,m,j
    return np.ascontiguousarray(v.transpose(0, 3, 2, 1, 4)).reshape(
        e, MH, 128, KD * 128
    )


def _swizzle_w2(w2_loc):
    # [E, H, D] -> [E, MD, G, 128p, 8*128] with w2s[e,m2,g,p,ki*128+j] = w2[e,(g*8+ki)*128+p,m2*128+j]
    e = w2_loc.shape[0]
    v = w2_loc.reshape(e, G, 8, 128, MD, 128)  # e,g,ki,p,m2,j
    return np.ascontiguousarray(v.transpose(0, 4, 1, 3, 2, 5)).reshape(
        e, MD, G, 128, 8 * 128
    )


def make_in_maps(x, w1, w2, b1, b2):
    bf16 = _np_bf16()
    x = np.asarray(x)
    w1 = np.asarray(w1)
    w2 = np.asarray(w2)
    b1 = np.asarray(b1)
    b2 = np.asarray(b2)
    b1c = np.ascontiguousarray(b1.reshape(MH, 128).T, dtype=np.float32)
    b2c = np.ascontiguousarray(b2.reshape(MD, 128).T, dtype=np.float32)
    in_maps = []
    for c in range(N_CORES):
        sl = slice(E_LOC * c, E_LOC * (c + 1))
        in_maps.append(
            {
                "xT": np.ascontiguousarray(x[sl].transpose(0, 2, 1)).astype(bf16),
                "w1s": _swizzle_w1(np.asarray(w1[sl])).astype(bf16),
                "w2s": _swizzle_w2(np.asarray(w2[sl])).astype(bf16),
                "b1c": b1c,
                "b2c": b2c,
            }
        )
    return in_maps


def kernel(x, w1, w2, b1, b2):
    from concourse import bass_utils

    nc = get_nc()
    in_maps = make_in_maps(x, w1, w2, b1, b2)
    res = bass_utils.run_bass_kernel_spmd(nc, in_maps, core_ids=list(range(N_CORES)))
    out = np.empty((E_FULL, N_TOK, D_DIM), dtype=np.float32)
    for c in range(N_CORES):
        out[E_LOC * c : E_LOC * (c + 1)] = res.results[c]["outT"].transpose(0, 2, 1)
    return out
